# revision 1
# baseline (speedup 1.0000x reference)
"""Trainium2 Bass kernel for nn_DecoderLayer (self-attn + cross-attn + FFN).

Sharding: 8 cores = (batch b in 0..3) x (query-half in 0..1). Each core
computes 512 query tokens of one batch element end-to-end; K/V projections
over the full source sequence are duplicated across the two halves of a
batch element, so no collectives are needed.

Per-core layout strategy:
  - activations kept feature-major (x^T: [D, tokens]) so every linear layer
    uses the weight matrix as stored (lhsT = W[k_chunk, out_chunk]).
  - attention scores computed transposed ([s, t]); softmax runs without
    max-subtraction (scores are O(1); masked entries are -1e20 -> exp = 0).
  - V computed token-major with an appended ones column so the ctx matmul
    also produces the softmax denominator; normalization is a per-partition
    tensor_scalar multiply on eviction.
  - LayerNorm runs token-major (bn_stats/bn_aggr along the free dim); the
    LN output is PE-transposed back to feature-major for the next stage.

Self-contained: hardcodes all shapes; no sibling imports.
"""

import numpy as np
import ml_dtypes
from contextlib import ExitStack

import concourse.bass as bass
import concourse.tile as tile
from concourse import bacc, mybir
from concourse.bass_utils import run_bass_kernel_spmd
from concourse.masks import make_identity

P = 128
LN_EPS = 1e-5

F32 = mybir.dt.float32

AF = mybir.ActivationFunctionType
ALU = mybir.AluOpType


def build_decoder_nc(D=1024, S=1024, TP=512, H=16, FF=4096, mm_dt=F32,
                     dram_mm_dt=None):
    """Build the per-core SPMD program.

    D: model dim; S: source seq len (= full T); TP: query tokens per core;
    H: heads (dh fixed 64); FF: ffn inner dim. mm_dt: dtype used for matmul
    operands (float32 or float32r). dram_mm_dt: dtype used to DECLARE the
    DRAM tensors that only feed matmuls (float32r trick); defaults to mm_dt.
    """
    dh = 64
    assert D % P == 0 and S % P == 0 and TP % P == 0 and FF % P == 0
    assert H * dh == D
    KC = D // P          # contraction chunks over D
    SB = S // P          # source blocks
    TB = TP // P         # query-token blocks
    NQ = TP              # free size of q/scores matmuls (<= 512)
    assert NQ <= 512
    VW = min(512, D)     # v-proj free width
    VH = D // VW
    OW = min(512, D)     # out-proj free width
    ODH = D // OW
    FFC = FF // P
    HPV = VW // dh       # heads per v-proj chunk

    if dram_mm_dt is None:
        dram_mm_dt = mm_dt

    nc = bacc.Bacc("TRN2", target_bir_lowering=False, debug=False)

    def din(name, shape, dt=F32):
        return nc.dram_tensor(name, shape, dt, kind="ExternalInput").ap()

    xqT = din("xqT", [D, TP], dram_mm_dt)     # queries slice, feature-major
    xq = din("xq", [TP, D])                   # queries slice, token-major
    xfT = din("xfT", [D, S], dram_mm_dt)      # full x[b], feature-major
    encT = din("encT", [D, S], dram_mm_dt)    # enc_out[b], feature-major
    m1T = din("m1T", [S, TP], mybir.dt.bfloat16)   # additive tgt mask [s,t]
    m2T = din("m2T", [S, TP], mybir.dt.bfloat16)   # additive src mask [s,t]
    wq1 = din("wq1", [D, D], dram_mm_dt)      # pre-scaled by dh**-0.5
    wkv1 = din("wkv1", [D, 2 * D], dram_mm_dt)
    wo1 = din("wo1", [D, D], dram_mm_dt)
    wq2 = din("wq2", [D, D], dram_mm_dt)
    wkv2 = din("wkv2", [D, 2 * D], dram_mm_dt)
    wo2 = din("wo2", [D, D], dram_mm_dt)
    w_in = din("w_in", [D, FF], dram_mm_dt)
    w_out = din("w_out", [FF, D], dram_mm_dt)
    out = nc.dram_tensor("out", [TP, D], F32, kind="ExternalOutput").ap()

    with tile.TileContext(nc) as tc:
        with ExitStack() as ctx:
            # ---- persistent pools ----
            consts = ctx.enter_context(tc.tile_pool(name="consts", bufs=1))
            p_res = ctx.enter_context(tc.tile_pool(name="p_res", bufs=3))
            p_wl = ctx.enter_context(tc.tile_pool(name="p_wl", bufs=3))
            p_wr = ctx.enter_context(tc.tile_pool(name="p_wr", bufs=8))
            p_stat = ctx.enter_context(tc.tile_pool(name="p_stat", bufs=10))
            p_msk = ctx.enter_context(tc.tile_pool(name="p_msk", bufs=2))
            pp_big = ctx.enter_context(
                tc.tile_pool(name="pp_big", bufs=4, space="PSUM"))
            pp_ctx = ctx.enter_context(
                tc.tile_pool(name="pp_ctx", bufs=2, space="PSUM"))
            pp_tr = ctx.enter_context(
                tc.tile_pool(name="pp_tr", bufs=2, space="PSUM"))

            ident = consts.tile([P, P], F32)
            make_identity(nc, ident)
            eps_t = consts.tile([P, 1], F32)
            nc.vector.memset(eps_t, LN_EPS)

            def layernorm(res, xout):
                """token-major LN: res/xout are [P, TB, D] tiles."""
                nsub = max(1, D // 512)
                w = D // nsub
                for tb in range(TB):
                    st = p_stat.tile([P, nsub, 6], F32, tag="lnst")
                    for g in range(nsub):
                        nc.vector.bn_stats(st[:, g, :],
                                           res[:, tb, g * w:(g + 1) * w])
                    mv = p_stat.tile([P, 2], F32, tag="lnmv")
                    nc.vector.bn_aggr(mv, st)
                    std = p_stat.tile([P, 1], F32, tag="lnstd")
                    nc.scalar.activation(std, mv[:, 1:2], AF.Sqrt, bias=eps_t)
                    rstd = p_stat.tile([P, 1], F32, tag="lnrstd")
                    nc.vector.reciprocal(rstd, std)
                    nc.vector.tensor_scalar(
                        out=xout[:, tb, :], in0=res[:, tb, :],
                        scalar1=mv[:, 0:1], scalar2=rstd,
                        op0=ALU.subtract, op1=ALU.mult)

            def transpose_to_fm(src, dstT):
                """src [P, TB, D] token-major -> dstT [P, KC, TP] feature-major."""
                for tb in range(TB):
                    for fc in range(KC):
                        ps = pp_tr.tile([P, P], F32)
                        nc.tensor.transpose(
                            ps, src[:, tb, fc * P:(fc + 1) * P], ident)
                        nc.vector.tensor_copy(
                            dstT[:, fc, tb * P:(tb + 1) * P], ps)

            def wl_col(w_d, c0, rows=D):
                """one DMA: [rows, P] weight column block as lhsT chunks
                [P, kc, P]."""
                wt = p_wl.tile([P, rows // P, P], mm_dt, tag="wl", name="wl")
                nc.sync.dma_start(
                    wt, w_d[:, c0:c0 + P].rearrange("(kc p) m -> p kc m", p=P))
                return wt

            def attn_stage(sctx, kvT_dram, q_src_T, wq_d, wkv_d, wo_d,
                           m_dram, res_in):
                """One attention block. Returns (x_out, x_outT-producer fn).

                kvT_dram: [D, S] feature-major dram AP for k/v source.
                q_src_T: either ("dram", AP [D, TP]) or ("tile", sbuf tile
                         [P, KC, TP]) for the feature-major query source.
                res_in: token-major [P, TB, D] residual source tile, or
                        ("dram", xq AP) for stage 1.
                """
                kvr = kvT_dram.rearrange("(c p) s -> p c s", p=P)

                # -- projections: kT, v (+ones), qT --
                kT = sctx.enter_context(
                    tc.tile_pool(name="kT", bufs=1)).tile([P, KC, S], mm_dt)
                vt = sctx.enter_context(
                    tc.tile_pool(name="vt", bufs=1)).tile(
                        [P, SB, H, dh + 1], F32)
                qT = sctx.enter_context(
                    tc.tile_pool(name="qT", bufs=1)).tile([P, KC, NQ], mm_dt)
                ones_c = consts.tile([P, H, 1], F32, tag="ones_c")
                nc.vector.memset(ones_c, 1.0)
                for sb in range(SB):
                    # rounding producer for the f32r ones column
                    nc.vector.tensor_copy(vt[:, sb, :, dh:dh + 1], ones_c)

                with tc.tile_pool(name="kv_src", bufs=1) as p_src, \
                        tc.tile_pool(name="q_src", bufs=3) as p_qsrc:
                    kvsrc = p_src.tile([P, KC, S], mm_dt)
                    for kc in range(KC):
                        for sh2 in range(2):
                            nc.sync.dma_start(
                                kvsrc[:, kc, sh2 * S // 2:(sh2 + 1) * S // 2],
                                kvr[:, kc, sh2 * S // 2:(sh2 + 1) * S // 2])

                    # kT: feature-major k = wk.T @ x^T
                    SH = S // 512 if S >= 512 else 1
                    SW = S // SH
                    for ofg in range(0, KC, 2):
                        ofs = range(ofg, min(ofg + 2, KC))
                        pss = {}
                        for of in ofs:
                            for sh in range(SH):
                                pss[(of, sh)] = pp_big.tile(
                                    [P, SW], F32, tag="ps", name="ps")
                        wts = {of: wl_col(wkv_d, of * P) for of in ofs}
                        for kc in range(KC):
                            for of in ofs:
                                for sh in range(SH):
                                    nc.tensor.matmul(
                                        pss[(of, sh)], wts[of][:, kc, :],
                                        kvsrc[:, kc, sh * SW:(sh + 1) * SW],
                                        start=(kc == 0), stop=(kc == KC - 1))
                        for of in ofs:
                            for sh in range(SH):
                                nc.scalar.copy(
                                    kT[:, of, sh * SW:(sh + 1) * SW],
                                    pss[(of, sh)])

                    # v token-major: v = x @ wv, heads interleaved, +1s col
                    SBG = 4 if SB % 4 == 0 else SB
                    for vh in range(VH):
                        for sbg in range(0, SB, SBG):
                            sbs = range(sbg, min(sbg + SBG, SB))
                            pss = {sb: pp_big.tile([P, VW], F32, tag="ps", name="ps")
                                   for sb in sbs}
                            for kc in range(KC):
                                wr = p_wr.tile([P, VW], mm_dt)
                                nc.sync.dma_start(
                                    wr, wkv_d[kc * P:(kc + 1) * P,
                                              D + vh * VW:D + (vh + 1) * VW])
                                for sb in sbs:
                                    nc.tensor.matmul(
                                        pss[sb],
                                        kvsrc[:, kc, sb * P:(sb + 1) * P],
                                        wr, start=(kc == 0),
                                        stop=(kc == KC - 1))
                            for sb in sbs:
                                nc.scalar.copy(
                                    vt[:, sb, vh * HPV:(vh + 1) * HPV, 0:dh],
                                    pss[sb].rearrange("p (h d) -> p h d",
                                                      d=dh))

                    # qT feature-major
                    if q_src_T[0] == "dram":
                        qsr = q_src_T[1].rearrange("(c p) t -> p c t", p=P)
                        qsrc = p_qsrc.tile([P, KC, NQ], mm_dt, tag="qsrc",
                                           bufs=1)
                        for kc in range(KC):
                            nc.sync.dma_start(qsrc[:, kc, :], qsr[:, kc, :])
                        qt_src = qsrc
                    else:
                        qt_src = q_src_T[1]
                    for ofg in range(0, KC, 2):
                        ofs = range(ofg, min(ofg + 2, KC))
                        pss = {of: pp_big.tile([P, NQ], F32, tag="ps",
                                               name="ps") for of in ofs}
                        wts = {of: wl_col(wq_d, of * P) for of in ofs}
                        for kc in range(KC):
                            for of in ofs:
                                nc.tensor.matmul(
                                    pss[of], wts[of][:, kc, :],
                                    qt_src[:, kc, :],
                                    start=(kc == 0), stop=(kc == KC - 1))
                        for of in ofs:
                            nc.scalar.copy(qT[:, of, :], pss[of])

                # -- per-head attention --
                ctxt = p_res.tile([P, TB, D], F32, tag="res")
                with tc.tile_pool(name="mT", bufs=1) as p_mT, \
                        tc.tile_pool(name="expp", bufs=20) as p_exp:
                    mT = p_mT.tile([P, SB, NQ], mybir.dt.bfloat16)
                    nc.sync.dma_start(
                        mT, m_dram.rearrange("(sb p) t -> p sb t", p=P))
                    # heads paired: consecutive K=64 score matmuls land on
                    # disjoint PE row groups (base_partition 0 / 64) and run
                    # concurrently.
                    for hp in range(0, H, 2):
                        pair = list(range(hp, min(hp + 2, H)))
                        ets = {}
                        for sb in range(SB):
                            for h in pair:
                                kc_h, ko = divmod(h * dh, P)
                                ps = pp_big.tile([P, NQ], F32, tag="ps",
                                                 name="ps")
                                nc.tensor.matmul(
                                    ps,
                                    kT[ko:ko + dh, kc_h, sb * P:(sb + 1) * P],
                                    qT[ko:ko + dh, kc_h, :],
                                    start=True, stop=True)
                                nc.vector.tensor_add(ps, ps, mT[:, sb, :])
                                et = p_exp.tile([P, NQ], F32, name="et")
                                nc.scalar.activation(et, ps, AF.Exp)
                                ets[(h, sb)] = et
                        for h in pair:
                            for tb in range(TB):
                                psc = pp_ctx.tile([P, dh + 1], F32, name="psc")
                                for sb in range(SB):
                                    nc.tensor.matmul(
                                        psc,
                                        ets[(h, sb)][:, tb * P:(tb + 1) * P],
                                        vt[:, sb, h, :],
                                        start=(sb == 0), stop=(sb == SB - 1))
                                rec = p_stat.tile([P, 1], F32, tag="rec",
                                                  name="rec")
                                nc.vector.reciprocal(rec, psc[:, dh:dh + 1])
                                nc.vector.tensor_scalar_mul(
                                    ctxt[:, tb, h * dh:(h + 1) * dh],
                                    in0=psc[:, 0:dh], scalar1=rec)

                # -- transpose ctx to feature-major --
                res = p_res.tile([P, TB, D], F32, tag="res")
                with tc.tile_pool(name="ctxT", bufs=1) as p_ctxT:
                    ctxT = p_ctxT.tile([P, KC, TP], mm_dt)
                    for tb in range(TB):
                        for fc in range(KC):
                            ps = pp_tr.tile([P, P], F32)
                            nc.tensor.transpose(
                                ps, ctxt[:, tb, fc * P:(fc + 1) * P], ident)
                            nc.vector.tensor_copy(
                                ctxT[:, fc, tb * P:(tb + 1) * P], ps)

                    # -- out-projection + residual --
                    if res_in[0] == "dram":
                        ri = p_res.tile([P, TB, D], F32, tag="res")
                        nc.sync.dma_start(
                            ri, res_in[1].rearrange("(tb p) d -> p tb d", p=P))
                        rsrc = ri
                    else:
                        rsrc = res_in[1]
                    for oh in range(ODH):
                        pss = {tb: pp_big.tile([P, OW], F32, tag="ps",
                                               name="ps") for tb in range(TB)}
                        for fc in range(KC):
                            wr = p_wr.tile([P, OW], mm_dt)
                            nc.sync.dma_start(
                                wr, wo_d[fc * P:(fc + 1) * P,
                                         oh * OW:(oh + 1) * OW])
                            for tb in range(TB):
                                nc.tensor.matmul(
                                    pss[tb], ctxT[:, fc, tb * P:(tb + 1) * P],
                                    wr, start=(fc == 0), stop=(fc == KC - 1))
                        for tb in range(TB):
                            nc.vector.tensor_add(
                                res[:, tb, oh * OW:(oh + 1) * OW], pss[tb],
                                rsrc[:, tb, oh * OW:(oh + 1) * OW])

                xo = p_res.tile([P, TB, D], F32, tag="res")
                layernorm(res, xo)
                xoT = p_res.tile([P, KC, TP], mm_dt, tag="res")
                transpose_to_fm(xo, xoT)
                return xo, xoT

            # ---------------- stage 1: self-attention ----------------
            with ExitStack() as s1:
                x1, x1T = attn_stage(s1, xfT, ("dram", xqT), wq1, wkv1, wo1,
                                     m1T, ("dram", xq))

            # ---------------- stage 2: cross-attention ----------------
            with ExitStack() as s2:
                x2, x2T = attn_stage(s2, encT, ("tile", x1T), wq2, wkv2, wo2,
                                     m2T, ("tile", x1))

            # ---------------- stage 3: FFN ----------------
            with tc.tile_pool(name="hT", bufs=1) as p_hT:
                hT = p_hT.tile([P, FFC, NQ], mm_dt)
                for ffc in range(FFC):
                    ps = pp_big.tile([P, NQ], F32, tag="ps", name="ps")
                    wt = wl_col(w_in, ffc * P)
                    for kc in range(KC):
                        nc.tensor.matmul(ps, wt[:, kc, :], x2T[:, kc, :],
                                         start=(kc == 0), stop=(kc == KC - 1))
                    nc.scalar.activation(hT[:, ffc, :], ps, AF.Relu)

                res3 = p_res.tile([P, TB, D], F32, tag="res")
                FFG = 8 if FFC % 8 == 0 else FFC
                for oh in range(ODH):
                    pss = {tb: pp_big.tile([P, OW], F32, tag="ps", name="ps")
                           for tb in range(TB)}
                    for ffg in range(0, FFC, FFG):
                        for ffc in range(ffg, min(ffg + FFG, FFC)):
                            wr = p_wr.tile([P, OW], mm_dt)
                            nc.sync.dma_start(
                                wr, w_out[ffc * P:(ffc + 1) * P,
                                          oh * OW:(oh + 1) * OW])
                            for tb in range(TB):
                                nc.tensor.matmul(
                                    pss[tb], hT[:, ffc, tb * P:(tb + 1) * P],
                                    wr, start=(ffc == 0),
                                    stop=(ffc == FFC - 1))
                    for tb in range(TB):
                        nc.vector.tensor_add(
                            res3[:, tb, oh * OW:(oh + 1) * OW], pss[tb],
                            x2[:, tb, oh * OW:(oh + 1) * OW])

                xout = p_res.tile([P, TB, D], F32, tag="res")
                layernorm(res3, xout)
                outr = out.rearrange("(tb p) d -> p tb d", p=P)
                for tb in range(TB):
                    nc.sync.dma_start(outr[:, tb, :], xout[:, tb, :])

    nc.compile()
    return nc


# ---------------------------------------------------------------------------
# host side
# ---------------------------------------------------------------------------

_NC_CACHE = {}


def _get_nc(key=("f32",)):
    if key not in _NC_CACHE:
        if key == ("f32",):
            _NC_CACHE[key] = build_decoder_nc(mm_dt=F32)
        elif key == ("f32r",):
            _NC_CACHE[key] = build_decoder_nc(mm_dt=mybir.dt.float32r)
        else:
            raise KeyError(key)
    return _NC_CACHE[key]


MM_KEY = ("f32r",)  # f32r: full-rate PE (4x fp32) at ~1e-4 matmul rel err


def _numpy_reference(x, enc_out, src_mask, tgt_mask, wq1, bq1, wkv1, bkv1,
                     wo1, bo1, wq2, bq2, wkv2, bkv2, wo2, bo2, w_in, b_in,
                     w_out, b_out, g0, be0, g1, be1, g2, be2):
    """Pure-numpy fallback (exact reference semantics)."""
    H, D = 16, 1024

    def ln(x, g, b):
        m = x.mean(-1, keepdims=True)
        v = ((x - m) ** 2).mean(-1, keepdims=True)
        return (x - m) / np.sqrt(v + LN_EPS) * g + b

    def attn(q_in, mem, mask, wq, bq, wkv, bkv, wo, bo):
        B, T, _ = q_in.shape
        S = mem.shape[1]
        dhl = D // H
        q = (q_in @ wq + bq).reshape(B, T, H, dhl) * (dhl ** -0.5)
        k, v = np.split(mem @ wkv + bkv, 2, axis=-1)
        k = k.reshape(B, S, H, dhl)
        v = v.reshape(B, S, H, dhl)
        sc = np.einsum('bthd,bshd->bhts', q, k)
        sc = np.where(mask[:, None, :, :], -1e20, sc)
        sc = sc - sc.max(-1, keepdims=True)
        w = np.exp(sc)
        w = w / w.sum(-1, keepdims=True)
        ctx = np.einsum('bhts,bshd->bthd', w, v).reshape(B, T, D)
        return ctx @ wo + bo

    y = attn(x, x, tgt_mask, wq1, bq1, wkv1, bkv1, wo1, bo1)
    x1 = ln(x + y, g0, be0)
    y = attn(x1, enc_out, src_mask, wq2, bq2, wkv2, bkv2, wo2, bo2)
    x2 = ln(x1 + y, g1, be1)
    y = np.maximum(x2 @ w_in + b_in, 0.0) @ w_out + b_out
    return ln(x2 + y, g2, be2)


def kernel(x, enc_out, src_mask, tgt_mask, wq1, bq1, wkv1, bkv1, wo1, bo1,
           wq2, bq2, wkv2, bkv2, wo2, bo2, w_in, b_in, w_out, b_out,
           g0, be0, g1, be1, g2, be2, _trace=False):
    x = np.asarray(x)
    args = dict(x=x, enc_out=np.asarray(enc_out),
                src_mask=np.asarray(src_mask), tgt_mask=np.asarray(tgt_mask),
                wq1=np.asarray(wq1), bq1=np.asarray(bq1),
                wkv1=np.asarray(wkv1), bkv1=np.asarray(bkv1),
                wo1=np.asarray(wo1), bo1=np.asarray(bo1),
                wq2=np.asarray(wq2), bq2=np.asarray(bq2),
                wkv2=np.asarray(wkv2), bkv2=np.asarray(bkv2),
                wo2=np.asarray(wo2), bo2=np.asarray(bo2),
                w_in=np.asarray(w_in), b_in=np.asarray(b_in),
                w_out=np.asarray(w_out), b_out=np.asarray(b_out),
                g0=np.asarray(g0), be0=np.asarray(be0),
                g1=np.asarray(g1), be1=np.asarray(be1),
                g2=np.asarray(g2), be2=np.asarray(be2))

    # the hardware kernel folds out zero biases / unit gains (true for this
    # problem's setup_inputs); anything else falls back to exact numpy.
    zeros = [args[k] for k in ("bq1", "bkv1", "bo1", "bq2", "bkv2", "bo2",
                               "b_in", "b_out", "be0", "be1", "be2")]
    ones = [args["g0"], args["g1"], args["g2"]]
    if any(np.any(z != 0) for z in zeros) or any(np.any(g != 1) for g in ones):
        res = _numpy_reference(**args)
        return res.astype(np.float32), x

    B, T, D = x.shape
    TP = T // 2
    dh = D // 16
    sc = np.float32(dh ** -0.5)

    in_maps = []
    for core in range(8):
        b, half = divmod(core, 2)
        t0 = half * TP
        xb = args["x"][b]
        xs = xb[t0:t0 + TP]
        in_maps.append({
            "xqT": np.ascontiguousarray(xs.T),
            "xq": np.ascontiguousarray(xs),
            "xfT": np.ascontiguousarray(xb.T),
            "encT": np.ascontiguousarray(args["enc_out"][b].T),
            "m1T": np.ascontiguousarray(
                np.where(args["tgt_mask"][b, t0:t0 + TP], np.float32(-1e20),
                         np.float32(0)).T).astype(ml_dtypes.bfloat16),
            "m2T": np.ascontiguousarray(
                np.where(args["src_mask"][b, t0:t0 + TP], np.float32(-1e20),
                         np.float32(0)).T).astype(ml_dtypes.bfloat16),
            "wq1": args["wq1"] * sc,
            "wkv1": args["wkv1"],
            "wo1": args["wo1"],
            "wq2": args["wq2"] * sc,
            "wkv2": args["wkv2"],
            "wo2": args["wo2"],
            "w_in": args["w_in"],
            "w_out": args["w_out"],
        })

    nc = _get_nc(MM_KEY)
    res = run_bass_kernel_spmd(nc, in_maps, core_ids=list(range(8)),
                               trace=_trace)
    outp = np.empty((B, T, D), np.float32)
    for core in range(8):
        b, half = divmod(core, 2)
        outp[b, half * TP:(half + 1) * TP] = res.results[core]["out"]
    if _trace:
        kernel.last_results = res
    return outp, x



# revision 17
# speedup vs baseline: 1.5396x; 1.5396x over previous
"""Trainium2 Bass kernel for nn_DecoderLayer (self-attn + cross-attn + FFN).

Sharding: 8 cores = (batch b in 0..3) x (query-half in 0..1). Each core
computes 512 query tokens of one batch element end-to-end; K/V projections
over the full source sequence are duplicated across the two halves of a
batch element, so no collectives are needed.

Dtype strategy (rel-err budget 2e-2):
  - fp8(e4m3) + DoubleRow matmuls (2 K-chunks per instruction) for the
    k/v/q projections, the ctx (weights@V) matmul, and the FFN first
    matmul. Weights are pre-scaled on the host into fp8-friendly ranges;
    the inverse scales fold into the exp() activation scale and into the
    bf16 weights of the following matmul.
  - bf16 for attention out-proj and FFN second matmul; fp8 operands (at
    bf16 rate) for the score matmul.
  - The additive attention mask is folded into the score PSUM
    accumulation group as an fp8e5-DoubleRow matmul against an
    identity/zero stationary pair, so no vector-engine mask add exists.
  - softmax runs without max-subtraction; exp() applies scale 1/128
    (undoing the fp8 weight scaling) and bias -4 (fp8e4 range safety);
    masked entries are -8192 pre-scale -> exp == 0. The +1s column of V
    provides the denominator.

Engine balance: exp/relu + pre-phase PSUM evictions on Activation;
in-phase evictions on GpSimd(Pool); transpose evictions, softmax
normalize, residual adds and LayerNorm on DVE. Stage-2 K/V projections
are emitted interleaved with stage-1 score groups as tensor-engine
filler while the Activation engine works through the exps.

Self-contained: hardcodes all shapes; no sibling imports.
"""

import numpy as np
import ml_dtypes
from contextlib import ExitStack

import concourse.bass as bass
import concourse.tile as tile
from concourse import bacc, mybir
from concourse.bass_utils import run_bass_kernel_spmd
from concourse.masks import make_identity

P = 128
LN_EPS = 1e-5

F32 = mybir.dt.float32
BF16 = mybir.dt.bfloat16
FP8 = mybir.dt.float8e4      # e4m3, max normal 240
FP8M = mybir.dt.float8e5     # e5m2, for masks / identity

AF = mybir.ActivationFunctionType
ALU = mybir.AluOpType
DR = mybir.MatmulPerfMode.DoubleRow

# host-side scale folding
QK_SCALE = 1.0 / 128.0       # wq x32 (incl dh^-0.5), wk x4 -> scores x128
EXP_BIAS = -4.0              # keeps exp() output inside fp8e4 range
MASK_VAL = -8192.0           # e5m2-exact; x1/128 - 4 => exp == 0


def build_decoder_nc(D=1024, S=1024, TP=512, H=16, FF=4096):
    dh = 64
    KC = D // P          # 8 contraction chunks over D
    SB = S // P          # 8 source blocks
    TB = TP // P         # 4 query-token blocks
    NQ = TP              # 512
    VH = 2               # v-proj column halves (512 each)
    VW = D // VH
    ODH = 2              # out-proj column halves
    OW = D // ODH
    FFC = FF // P        # 32
    HPV = VW // dh       # 8 heads per v half

    nc = bacc.Bacc("TRN2", target_bir_lowering=False, debug=False)

    def din(name, shape, dt):
        return nc.dram_tensor(name, shape, dt, kind="ExternalInput").ap()

    xfT8 = din("xfT8", [D, S], FP8)          # x[b]^T (kv source, stage 1)
    xqT8 = din("xqT8", [D, TP], FP8)         # query-slice^T (q source)
    xtok = din("xtok", [TP, D], BF16)        # query-slice (residual)
    encT8 = din("encT8", [D, S], FP8)        # enc_out[b]^T (kv source, st 2)
    m8_1 = din("m8_1", [P, SB + 1, NQ], FP8M)
    m8_2 = din("m8_2", [P, SB + 1, NQ], FP8M)
    wk1 = din("wk1", [P, KC, D], FP8)        # x4, lhsT layout
    wq1 = din("wq1", [P, KC, D], FP8)        # x32 (incl dh^-0.5)
    wv1 = din("wv1", [P, KC, D], FP8)        # x4, moving layout
    wo1 = din("wo1", [P, KC, D], BF16)       # /4
    wk2 = din("wk2", [P, KC, D], FP8)
    wq2 = din("wq2", [P, KC, D], FP8)
    wv2 = din("wv2", [P, KC, D], FP8)
    wo2 = din("wo2", [P, KC, D], BF16)
    w8in = din("w8in", [P, FFC, KC, P], FP8)  # x4, per-ffc lhsT chunks
    wout = din("wout", [P, FFC, D], BF16)    # /4
    out = nc.dram_tensor("out", [TP, D], F32, kind="ExternalOutput").ap()

    with tile.TileContext(nc) as tc:
        with ExitStack() as ctx:
            consts = ctx.enter_context(tc.tile_pool(name="consts", bufs=1))
            p_stat = ctx.enter_context(tc.tile_pool(name="p_stat", bufs=10))
            p_res = ctx.enter_context(tc.tile_pool(name="p_res", bufs=1))
            p_et = ctx.enter_context(tc.tile_pool(name="p_et", bufs=2))
            p_wst = ctx.enter_context(tc.tile_pool(name="p_wst", bufs=4))
            pp_sc = ctx.enter_context(
                tc.tile_pool(name="pp_sc", bufs=4, space="PSUM"))
            pp_fill = ctx.enter_context(
                tc.tile_pool(name="pp_fill", bufs=2, space="PSUM"))
            pp_ctx = ctx.enter_context(
                tc.tile_pool(name="pp_ctx", bufs=2, space="PSUM"))

            identf = consts.tile([P, P], F32)
            make_identity(nc, identf)
            identb = consts.tile([P, P], BF16)
            nc.gpsimd.tensor_copy(identb, identf)
            idz8 = consts.tile([P, 2, P], FP8M)
            nc.gpsimd.memset(idz8, 0.0)
            nc.gpsimd.tensor_copy(idz8[:, 0, :], identf)
            eps_t = consts.tile([P, 1], F32)
            nc.vector.memset(eps_t, LN_EPS)
            ebias_t = consts.tile([P, 1], F32)
            nc.vector.memset(ebias_t, EXP_BIAS)

            # ---------------- helpers -------------------------------------
            def dr_group(ps, wt, src, of, n0, n1):
                """ps = (w col-block of).T @ src[:, :, n0:n1] via DR pairs."""
                for kcp in range(KC // 2):
                    nc.tensor.matmul(
                        ps, wt[:, 2 * kcp:2 * kcp + 2, of * P:(of + 1) * P],
                        src[:, 2 * kcp:2 * kcp + 2, n0:n1],
                        start=(kcp == 0), stop=(kcp == KC // 2 - 1),
                        perf_mode=DR)

            def v_group(vt, wvt, kvs, vh, sbg, evict):
                """token-major v projection, 2 source blocks at a time."""
                pss = []
                for sb in (sbg, sbg + 1):
                    ps = pp_fill.tile([P, VW], F32, tag="psf", name="psf")
                    for kcp in range(KC // 2):
                        nc.tensor.matmul(
                            ps, kvs[:, 2 * kcp:2 * kcp + 2,
                                    sb * P:(sb + 1) * P],
                            wvt[:, 2 * kcp:2 * kcp + 2, vh * VW:(vh + 1) * VW],
                            start=(kcp == 0), stop=(kcp == KC // 2 - 1),
                            perf_mode=DR)
                    pss.append(ps)
                for i, sb in enumerate((sbg, sbg + 1)):
                    evict(vt[:, sb, vh * HPV:(vh + 1) * HPV, 0:dh],
                          pss[i].rearrange("p (h d) -> p h d", d=dh))

            def score_head(kT, qT, mt, et, h):
                kc_h, ko = divmod(h * dh, P)
                for sb in range(SB):
                    ps = pp_sc.tile([P, NQ], F32, tag="ps", name="ps")
                    nc.tensor.matmul(
                        ps, kT[ko:ko + dh, kc_h, sb * P:(sb + 1) * P],
                        qT[ko:ko + dh, kc_h, :],
                        start=True, stop=False)
                    nc.tensor.matmul(
                        ps, idz8, mt[:, sb:sb + 2, :],
                        start=False, stop=True, perf_mode=DR)
                    nc.scalar.activation(et[:, sb, :], ps, AF.Exp,
                                         bias=ebias_t, scale=QK_SCALE)

            def ctx_head(et, vt, ctxt, h):
                for tb in range(TB):
                    psc = pp_ctx.tile([P, 512], F32, tag="psc", name="psc")
                    for sbp in range(SB // 2):
                        nc.tensor.matmul(
                            psc[:, 0:dh + 1],
                            et[:, 2 * sbp:2 * sbp + 2, tb * P:(tb + 1) * P],
                            vt[:, 2 * sbp:2 * sbp + 2, h, :],
                            start=(sbp == 0), stop=(sbp == SB // 2 - 1),
                            perf_mode=DR)
                    rec = p_stat.tile([P, 1], F32, tag="rec", name="rec")
                    nc.vector.reciprocal(rec, psc[:, dh:dh + 1])
                    nc.vector.tensor_scalar_mul(
                        ctxt[:, tb, h * dh:(h + 1) * dh],
                        in0=psc[:, 0:dh], scalar1=rec)

            def transpose_tm_to_fm(src, dstT):
                """src [P, TB, D] token-major -> dstT [P, KC, TP]."""
                for tb in range(TB):
                    for fc in range(KC):
                        ps = pp_sc.tile([P, P], BF16, tag="ps", name="ps")
                        nc.tensor.transpose(
                            ps, src[:, tb, fc * P:(fc + 1) * P], identb)
                        nc.vector.tensor_copy(
                            dstT[:, fc, tb * P:(tb + 1) * P], ps)

            def alloc_8psums():
                pss = {}
                for i, (oh, tb) in enumerate(
                        [(o, t) for o in range(ODH) for t in range(TB)]):
                    if i < 4:
                        pss[(oh, tb)] = pp_sc.tile([P, OW], F32, tag="ps",
                                                   name="ps")
                    elif i < 6:
                        pss[(oh, tb)] = pp_fill.tile([P, OW], F32, tag="psf",
                                                     name="psf")
                    else:
                        pss[(oh, tb)] = pp_ctx.tile([P, OW], F32, tag="psc",
                                                    name="psc")
                return pss

            def residual_adds(pss, rsrc, res):
                for oh in range(ODH):
                    for tb in range(TB):
                        nc.vector.tensor_tensor(
                            res[:, tb, oh * OW:(oh + 1) * OW], pss[(oh, tb)],
                            rsrc[:, tb, oh * OW:(oh + 1) * OW], ALU.add)

            def out_proj_residual(ctxT, wo_d, rsrc, res):
                """res = ctxT.T @ wo + rsrc (token-major, bf16); wo is
                streamed from DRAM per contraction chunk."""
                wts = []
                for fc in range(3):
                    wt = p_wst.tile([P, D], BF16, tag="wo", name="wo")
                    nc.sync.dma_start(wt, wo_d[:, fc, :])
                    wts.append(wt)
                pss = alloc_8psums()
                for fc in range(KC):
                    if fc + 3 < KC:
                        wt = p_wst.tile([P, D], BF16, tag="wo", name="wo")
                        nc.sync.dma_start(wt, wo_d[:, fc + 3, :])
                        wts.append(wt)
                    for oh in range(ODH):
                        for tb in range(TB):
                            nc.tensor.matmul(
                                pss[(oh, tb)],
                                ctxT[:, fc, tb * P:(tb + 1) * P],
                                wts[fc][:, oh * OW:(oh + 1) * OW],
                                start=(fc == 0), stop=(fc == KC - 1))
                residual_adds(pss, rsrc, res)

            def layernorm(res, xout):
                """token-major LN over D: res/xout [P, TB, D]."""
                for tb in range(TB):
                    st = p_stat.tile([P, 2, 6], F32, tag="lnst", name="lnst")
                    for g in range(2):
                        nc.vector.bn_stats(st[:, g, :],
                                           res[:, tb, g * 512:(g + 1) * 512])
                    mv = p_stat.tile([P, 2], F32, tag="lnmv", name="lnmv")
                    nc.vector.bn_aggr(mv, st)
                    std = p_stat.tile([P, 1], F32, tag="lnstd", name="lnstd")
                    nc.scalar.activation(std, mv[:, 1:2], AF.Sqrt, bias=eps_t)
                    rstd = p_stat.tile([P, 1], F32, tag="lnrstd",
                                       name="lnrstd")
                    nc.vector.reciprocal(rstd, std)
                    nc.vector.tensor_scalar(
                        out=xout[:, tb, :], in0=res[:, tb, :],
                        scalar1=mv[:, 0:1], scalar2=rstd,
                        op0=ALU.subtract, op1=ALU.mult)

            def act_evict(dst, ps):
                nc.scalar.copy(dst, ps)

            def pool_evict(dst, ps):
                nc.gpsimd.tensor_copy(dst, ps)

            # residual-chain tiles (outer, tag-rotated)
            xtok_t = p_res.tile([P, TB, D], BF16, name="xtok_t", bufs=1)
            ctxt1 = p_res.tile([P, TB, D], BF16, tag="ctxt", name="ctxt",
                               bufs=1)
            ctxT1 = p_res.tile([P, KC, TP], BF16, tag="ctxT", name="ctxT",
                               bufs=1)

            pC = ctx.enter_context(tc.tile_pool(name="pC", bufs=1))
            with tc.tile_pool(name="pB", bufs=1) as pB:
                with tc.tile_pool(name="pA", bufs=1) as pA:
                    kvs1 = pA.tile([P, KC, S], FP8, name="kvs1")
                    nc.sync.dma_start(
                        kvs1, xfT8.rearrange("(kc p) s -> p kc s", p=P))
                    wk1t = pA.tile([P, KC, D], FP8, name="wk1t")
                    nc.sync.dma_start(wk1t, wk1)
                    wq1t = pA.tile([P, KC, D], FP8, name="wq1t")
                    nc.sync.dma_start(wq1t, wq1)
                    qs1 = pA.tile([P, KC, NQ], FP8, name="qs1")
                    nc.sync.dma_start(
                        qs1, xqT8.rearrange("(kc p) t -> p kc t", p=P))
                    m1t = pA.tile([P, SB + 1, NQ], FP8M, name="m1t")
                    nc.sync.dma_start(m1t, m8_1)
                    wv1t = pA.tile([P, KC, D], FP8, name="wv1t")
                    nc.sync.dma_start(wv1t, wv1)
                    kvs2 = pB.tile([P, KC, S], FP8, name="kvs2")
                    nc.sync.dma_start(
                        kvs2, encT8.rearrange("(kc p) s -> p kc s", p=P))
                    wk2t = pB.tile([P, KC, D], FP8, name="wk2t")
                    nc.sync.dma_start(wk2t, wk2)
                    wv2t = pB.tile([P, KC, D], FP8, name="wv2t")
                    nc.sync.dma_start(wv2t, wv2)
                    nc.sync.dma_start(
                        xtok_t, xtok.rearrange("(tb p) d -> p tb d", p=P))

                    # ---- stage 1 projections ---------------------------
                    kT1 = pA.tile([P, KC, S], FP8, name="kT1")
                    qT1 = pA.tile([P, KC, NQ], FP8, name="qT1")
                    vt1 = pA.tile([P, SB, H, dh + 1], FP8, name="vt1")
                    nc.gpsimd.memset(vt1[:, :, :, dh:dh + 1], 1.0)

                    for of in range(KC):
                        for sh in range(2):
                            ps = pp_sc.tile([P, 512], F32, tag="ps",
                                            name="ps")
                            dr_group(ps, wk1t, kvs1, of, sh * 512,
                                     (sh + 1) * 512)
                            act_evict(kT1[:, of, sh * 512:(sh + 1) * 512],
                                      ps)
                    for of in range(KC):
                        ps = pp_sc.tile([P, NQ], F32, tag="ps", name="ps")
                        dr_group(ps, wq1t, qs1, of, 0, NQ)
                        act_evict(qT1[:, of, :], ps)

                    # ---- stage 1 score phase + fillers -----------------
                    kT2 = pB.tile([P, KC, S], FP8, name="kT2")
                    vt2 = pB.tile([P, SB, H, dh + 1], FP8, name="vt2")
                    nc.gpsimd.memset(vt2[:, :, :, dh:dh + 1], 1.0)

                    fillers = []
                    for vh in range(VH):
                        for sbg in range(0, SB, 2):
                            fillers.append(
                                lambda vh=vh, sbg=sbg: v_group(
                                    vt1, wv1t, kvs1, vh, sbg, pool_evict))
                    for of in range(KC):
                        for sh in range(2):
                            def k2_chunk(of=of, sh=sh):
                                ps = pp_fill.tile([P, 512], F32, tag="psf",
                                                  name="psf")
                                dr_group(ps, wk2t, kvs2, of, sh * 512,
                                         (sh + 1) * 512)
                                pool_evict(
                                    kT2[:, of, sh * 512:(sh + 1) * 512], ps)
                            fillers.append(k2_chunk)
                    fi = 0

                    ets = {}
                    for h in range(H):
                        ets[h] = p_et.tile([P, SB, NQ], FP8, tag="et",
                                           name="et")
                        score_head(kT1, qT1, m1t, ets[h], h)
                        for _ in range(2 if h % 2 == 0 else 1):
                            if fi < len(fillers):
                                fillers[fi]()
                                fi += 1
                        if h >= 1:
                            ctx_head(ets[h - 1], vt1, ctxt1, h - 1)
                            ets.pop(h - 1)
                    while fi < len(fillers):
                        fillers[fi]()
                        fi += 1
                    ctx_head(ets[H - 1], vt1, ctxt1, H - 1)
                    ets.clear()

                # pA closed: stage-1 k/q/v tiles + sources freed
                # ---- stage 1 out-proj + LN -----------------------------
                transpose_tm_to_fm(ctxt1, ctxT1)
                res1 = p_res.tile([P, TB, D], BF16, tag="res", name="res",
                                  bufs=2)
                out_proj_residual(ctxT1, wo1, xtok_t, res1)
                x1 = p_res.tile([P, TB, D], BF16, tag="res", name="res",
                                bufs=2)
                layernorm(res1, x1)
                x1T8 = pB.tile([P, KC, TP], FP8, name="x1T8")
                transpose_tm_to_fm(x1, x1T8)

                # ---- stage 2 -------------------------------------------
                wq2t = pB.tile([P, KC, D], FP8, name="wq2t")
                nc.sync.dma_start(wq2t, wq2)
                m2t = pB.tile([P, SB + 1, NQ], FP8M, name="m2t")
                nc.sync.dma_start(m2t, m8_2)

                qT2 = pB.tile([P, KC, NQ], FP8, name="qT2")
                for of in range(KC):
                    ps = pp_sc.tile([P, NQ], F32, tag="ps", name="ps")
                    dr_group(ps, wq2t, x1T8, of, 0, NQ)
                    act_evict(qT2[:, of, :], ps)

                ctxt2 = p_res.tile([P, TB, D], BF16, tag="ctxt", name="ctxt",
                                   bufs=1)
                fillers2 = []
                for vh in range(VH):
                    for sbg in range(0, SB, 2):
                        fillers2.append(
                            lambda vh=vh, sbg=sbg: v_group(
                                vt2, wv2t, kvs2, vh, sbg, pool_evict))
                fi2 = 0
                ets2 = {}
                for h in range(H):
                    ets2[h] = p_et.tile([P, SB, NQ], FP8, tag="et",
                                        name="et")
                    score_head(kT2, qT2, m2t, ets2[h], h)
                    if fi2 < len(fillers2) and h % 2 == 0:
                        fillers2[fi2]()
                        fi2 += 1
                    if h >= 1:
                        ctx_head(ets2[h - 1], vt2, ctxt2, h - 1)
                        ets2.pop(h - 1)
                while fi2 < len(fillers2):
                    fillers2[fi2]()
                    fi2 += 1
                ctx_head(ets2[H - 1], vt2, ctxt2, H - 1)
                ets2.clear()

                ctxT2 = p_res.tile([P, KC, TP], BF16, tag="ctxT",
                                   name="ctxT", bufs=1)
                transpose_tm_to_fm(ctxt2, ctxT2)
                res2 = p_res.tile([P, TB, D], BF16, tag="res", name="res",
                                  bufs=2)
                out_proj_residual(ctxT2, wo2, x1, res2)
                x2 = p_res.tile([P, TB, D], BF16, tag="res", name="res",
                                bufs=2)
                layernorm(res2, x2)
                x2T8 = pC.tile([P, KC, TP], FP8, name="x2T8")
                transpose_tm_to_fm(x2, x2T8)

            # pB closed: stage-2 tiles freed
            # ---- FFN ---------------------------------------------------
            p_hT = ctx.enter_context(tc.tile_pool(name="p_hT", bufs=1))
            hT = p_hT.tile([P, FFC, NQ], BF16, name="hT")
            with tc.tile_pool(name="p_win", bufs=3) as p_win:
                wps = []
                for fp in range(2):
                    wp = p_win.tile([P, 2, KC, P], FP8, tag="win",
                                    name="win")
                    nc.sync.dma_start(wp, w8in[:, 2 * fp:2 * fp + 2, :, :])
                    wps.append(wp)
                for fp in range(FFC // 2):
                    if fp + 2 < FFC // 2:
                        wp = p_win.tile([P, 2, KC, P], FP8, tag="win",
                                        name="win")
                        nc.sync.dma_start(
                            wp, w8in[:, 2 * fp + 4:2 * fp + 6, :, :])
                        wps.append(wp)
                    for f in range(2):
                        ffc = 2 * fp + f
                        ps = pp_sc.tile([P, NQ], F32, tag="ps", name="ps")
                        for kcp in range(KC // 2):
                            nc.tensor.matmul(
                                ps,
                                wps[fp][:, f, 2 * kcp:2 * kcp + 2, :],
                                x2T8[:, 2 * kcp:2 * kcp + 2, :],
                                start=(kcp == 0), stop=(kcp == KC // 2 - 1),
                                perf_mode=DR)
                        nc.scalar.activation(hT[:, ffc, :], ps, AF.Relu)

            res3 = p_res.tile([P, TB, D], BF16, tag="res", name="res",
                              bufs=2)
            with tc.tile_pool(name="p_wout", bufs=3) as p_wout:
                pss = alloc_8psums()
                for q in range(FFC // 4):
                    wqt = p_wout.tile([P, 4, D], BF16, tag="wout",
                                      name="wout")
                    nc.sync.dma_start(wqt, wout[:, 4 * q:4 * q + 4, :])
                    for f in range(4):
                        ffc = 4 * q + f
                        for oh in range(ODH):
                            for tb in range(TB):
                                nc.tensor.matmul(
                                    pss[(oh, tb)],
                                    hT[:, ffc, tb * P:(tb + 1) * P],
                                    wqt[:, f, oh * OW:(oh + 1) * OW],
                                    start=(ffc == 0), stop=(ffc == FFC - 1))
                residual_adds(pss, x2, res3)

            outr = out.rearrange("(tb p) d -> p tb d", p=P)
            for tb in range(TB):
                st = p_stat.tile([P, 2, 6], F32, tag="lnst", name="lnst")
                for g in range(2):
                    nc.vector.bn_stats(st[:, g, :],
                                       res3[:, tb, g * 512:(g + 1) * 512])
                mv = p_stat.tile([P, 2], F32, tag="lnmv", name="lnmv")
                nc.vector.bn_aggr(mv, st)
                std = p_stat.tile([P, 1], F32, tag="lnstd", name="lnstd")
                nc.scalar.activation(std, mv[:, 1:2], AF.Sqrt, bias=eps_t)
                rstd = p_stat.tile([P, 1], F32, tag="lnrstd", name="lnrstd")
                nc.vector.reciprocal(rstd, std)
                xo = p_res.tile([P, D], F32, tag="xo", name="xo", bufs=2)
                nc.vector.tensor_scalar(
                    out=xo, in0=res3[:, tb, :],
                    scalar1=mv[:, 0:1], scalar2=rstd,
                    op0=ALU.subtract, op1=ALU.mult)
                nc.sync.dma_start(outr[:, tb, :], xo)

    nc.compile()
    return nc


# ---------------------------------------------------------------------------
# host side
# ---------------------------------------------------------------------------

_NC_CACHE = {}


def _get_nc(key="v2"):
    if key not in _NC_CACHE:
        _NC_CACHE[key] = build_decoder_nc()
    return _NC_CACHE[key]


MM_KEY = "v2"

E4 = ml_dtypes.float8_e4m3
E5 = ml_dtypes.float8_e5m2
BF = ml_dtypes.bfloat16


def _lhsT_layout(w):
    """[D, M] -> [P, D//P, M] (row chunks onto partitions)."""
    Dd, M = w.shape
    return np.ascontiguousarray(
        w.reshape(Dd // P, P, M).transpose(1, 0, 2))


def _numpy_reference(x, enc_out, src_mask, tgt_mask, wq1, bq1, wkv1, bkv1,
                     wo1, bo1, wq2, bq2, wkv2, bkv2, wo2, bo2, w_in, b_in,
                     w_out, b_out, g0, be0, g1, be1, g2, be2):
    """Pure-numpy fallback (exact reference semantics)."""
    H, D = 16, 1024

    def ln(x, g, b):
        m = x.mean(-1, keepdims=True)
        v = ((x - m) ** 2).mean(-1, keepdims=True)
        return (x - m) / np.sqrt(v + LN_EPS) * g + b

    def attn(q_in, mem, mask, wq, bq, wkv, bkv, wo, bo):
        B, T, _ = q_in.shape
        S = mem.shape[1]
        dhl = D // H
        q = (q_in @ wq + bq).reshape(B, T, H, dhl) * (dhl ** -0.5)
        k, v = np.split(mem @ wkv + bkv, 2, axis=-1)
        k = k.reshape(B, S, H, dhl)
        v = v.reshape(B, S, H, dhl)
        sc = np.einsum('bthd,bshd->bhts', q, k)
        sc = np.where(mask[:, None, :, :], -1e20, sc)
        sc = sc - sc.max(-1, keepdims=True)
        w = np.exp(sc)
        w = w / w.sum(-1, keepdims=True)
        ctx = np.einsum('bhts,bshd->bthd', w, v).reshape(B, T, D)
        return ctx @ wo + bo

    y = attn(x, x, tgt_mask, wq1, bq1, wkv1, bkv1, wo1, bo1)
    x1 = ln(x + y, g0, be0)
    y = attn(x1, enc_out, src_mask, wq2, bq2, wkv2, bkv2, wo2, bo2)
    x2 = ln(x1 + y, g1, be1)
    y = np.maximum(x2 @ w_in + b_in, 0.0) @ w_out + b_out
    return ln(x2 + y, g2, be2)


def kernel(x, enc_out, src_mask, tgt_mask, wq1, bq1, wkv1, bkv1, wo1, bo1,
           wq2, bq2, wkv2, bkv2, wo2, bo2, w_in, b_in, w_out, b_out,
           g0, be0, g1, be1, g2, be2, _trace=False):
    x = np.asarray(x)
    args = dict(x=x, enc_out=np.asarray(enc_out),
                src_mask=np.asarray(src_mask), tgt_mask=np.asarray(tgt_mask),
                wq1=np.asarray(wq1), bq1=np.asarray(bq1),
                wkv1=np.asarray(wkv1), bkv1=np.asarray(bkv1),
                wo1=np.asarray(wo1), bo1=np.asarray(bo1),
                wq2=np.asarray(wq2), bq2=np.asarray(bq2),
                wkv2=np.asarray(wkv2), bkv2=np.asarray(bkv2),
                wo2=np.asarray(wo2), bo2=np.asarray(bo2),
                w_in=np.asarray(w_in), b_in=np.asarray(b_in),
                w_out=np.asarray(w_out), b_out=np.asarray(b_out),
                g0=np.asarray(g0), be0=np.asarray(be0),
                g1=np.asarray(g1), be1=np.asarray(be1),
                g2=np.asarray(g2), be2=np.asarray(be2))

    # the hardware kernel folds out zero biases / unit gains (true for this
    # problem's setup_inputs); anything else falls back to exact numpy.
    zeros = [args[k] for k in ("bq1", "bkv1", "bo1", "bq2", "bkv2", "bo2",
                               "b_in", "b_out", "be0", "be1", "be2")]
    ones = [args["g0"], args["g1"], args["g2"]]
    if any(np.any(z != 0) for z in zeros) or any(np.any(g != 1) for g in ones):
        res = _numpy_reference(**args)
        return res.astype(np.float32), x

    B, T, D = x.shape
    TP = T // 2
    dh = D // 16
    sc = np.float32(dh ** -0.5)

    # shared weight conversions (lhsT layouts + fp8/bf16 scale folding)
    wk_1 = _lhsT_layout(args["wkv1"][:, :D] * 4.0).astype(E4)
    wv_1 = _lhsT_layout(args["wkv1"][:, D:] * 4.0).astype(E4)
    wq_1 = _lhsT_layout(args["wq1"] * (sc * 32.0)).astype(E4)
    wo_1 = _lhsT_layout(args["wo1"] * 0.25).astype(BF)
    wk_2 = _lhsT_layout(args["wkv2"][:, :D] * 4.0).astype(E4)
    wv_2 = _lhsT_layout(args["wkv2"][:, D:] * 4.0).astype(E4)
    wq_2 = _lhsT_layout(args["wq2"] * (sc * 32.0)).astype(E4)
    wo_2 = _lhsT_layout(args["wo2"] * 0.25).astype(BF)
    KC, FFC = D // P, args["w_in"].shape[1] // P
    w8in = np.ascontiguousarray(
        (args["w_in"] * 4.0).reshape(KC, P, FFC, P)
        .transpose(1, 2, 0, 3)).astype(E4)
    wout = _lhsT_layout(args["w_out"] * 0.25).astype(BF)

    SBp1, NQ = T // P + 1, TP

    def mk_mask(mask_slice):
        """[TP, S] bool -> [P, SB+1, NQ] e5m2 additive (transposed)."""
        S = mask_slice.shape[1]
        mT = np.where(mask_slice.T, np.float32(MASK_VAL), np.float32(0.0))
        m = np.zeros((P, SBp1, NQ), np.float32)
        m[:, :S // P, :] = mT.reshape(S // P, P, NQ).transpose(1, 0, 2)
        return m.astype(E5)

    in_maps = []
    for core in range(8):
        b, half = divmod(core, 2)
        t0 = half * TP
        xb = args["x"][b]
        xs = xb[t0:t0 + TP]
        in_maps.append({
            "xfT8": np.ascontiguousarray(xb.T).astype(E4),
            "xqT8": np.ascontiguousarray(xs.T).astype(E4),
            "xtok": np.ascontiguousarray(xs).astype(BF),
            "encT8": np.ascontiguousarray(args["enc_out"][b].T).astype(E4),
            "m8_1": mk_mask(args["tgt_mask"][b, t0:t0 + TP]),
            "m8_2": mk_mask(args["src_mask"][b, t0:t0 + TP]),
            "wk1": wk_1, "wq1": wq_1, "wv1": wv_1, "wo1": wo_1,
            "wk2": wk_2, "wq2": wq_2, "wv2": wv_2, "wo2": wo_2,
            "w8in": w8in, "wout": wout,
        })

    nc = _get_nc(MM_KEY)
    res = run_bass_kernel_spmd(nc, in_maps, core_ids=list(range(8)),
                               trace=_trace)
    outp = np.empty((B, T, D), np.float32)
    for core in range(8):
        b, half = divmod(core, 2)
        outp[b, half * TP:(half + 1) * TP] = res.results[core]["out"]
    if _trace:
        kernel.last_results = res
    return outp, x
